# revision 11
# baseline (speedup 1.0000x reference)
"""Trainium2 Bass kernel for nn_DeepRecursiveNLM.

Math (per batch row b, per channel d):
    h1[b,d,h] = relu(sum_m x[b,d,m] * W1[m,h,d] + b1[h,d])      m over last 15 history
    h2[b,d,h] = relu(h1[b,d,h] * W2[h,d] + b2[h,d])
    out[b,d]  = sum_h h2[b,d,h] * W3[h,d] + b3[d]
where W2/W3 derive from W1 via softmax-contraction + SVD spectral ops (tiny,
computed on host in numpy; the heavy batch computation runs on device).

Strategy: pure data parallel over batch across 8 NeuronCores (4096 rows/core).
Host prep (layout/dtype only, no model FLOPs):
  - keep only the used history half, cast to fp8 e3m4 (~1.3% quantization,
    well inside the 2e-2 gate; N(0,1) fits e3m4 range, and the PE handles
    e3m4 denormals exactly -- verified on HW), pre-transposed so the
    contraction dim (d8,m)=120 sits on SBUF partitions, supertile-major so
    each supertile's DMA is one contiguous block with 8KB descriptor runs
  - fold W1 into a per-group block-diagonal bf16 stationary (120x32);
    mixed fp8e3-moving x bf16-stationary matmul is exact on HW
Device, per super-tile of 512 batch rows:
  - one contiguous DMA loads (120, 16 groups, 512) fp8 on the sync HWDGE
    queue (consts go on the scalar queue so input streams immediately)
  - mm1: per pack of 4 groups, 4 matmuls write disjoint 32-partition
    quadrants of one PSUM bank (PE tile_position col-offset)
  - zero-bias fast path: h2 = relu(psum * max(W2,0)) in ONE elementwise op,
    alternating VectorE/ScalarE per pack (general 2-pass path kept for
    nonzero biases)
  - mm2 uses h2 as the stationary operand so output lands as (b, d) in one
    full PSUM bank per supertile; single copy (+b3 if nonzero) to bf16 SBUF;
    one DMA per supertile to a scrambled DRAM layout host un-shuffles.
"""

import numpy as np
import ml_dtypes

import concourse.bass as bass
import concourse.tile as tile
from concourse import bacc, mybir
from concourse.bass_utils import run_bass_kernel_spmd

N_CORES = 8
B, D, HIST = 32768, 128, 30
M, H, RANK = 15, 4, 8
BC = B // N_CORES          # 4096 batch rows per core
BLK = 128                  # batch rows per block (= PSUM partitions for mm2)
NBLK_ST = 4                # blocks per super-tile
STB = BLK * NBLK_ST        # 512
NST = BC // STB            # 8 super-tiles per core
GS = 8                     # d's per group
NG = D // GS               # 16 groups
NGP = 4                    # groups per pack
NP = NG // NGP             # 4 packs
M15 = 15
KG = GS * M15              # contraction rows per d-group = 120
F32 = mybir.dt.float32
BF16 = mybir.dt.bfloat16
FP8E3 = mybir.dt.float8e3
RELU = mybir.ActivationFunctionType.Relu
BF = ml_dtypes.bfloat16
F8 = ml_dtypes.float8_e3m4

_COMPILED = {}


def _build_nc(zero_b12, zero_b3):
    nc = bacc.Bacc("TRN2", target_bir_lowering=False, debug=False,
                   num_devices=N_CORES)
    xt = nc.declare_dram_parameter("xt", [NST, KG, NG, STB], FP8E3, isOutput=False)
    w1bd = nc.declare_dram_parameter("w1bd", [KG, NG, 32], BF16, isOutput=False)
    w3bd = nc.declare_dram_parameter("w3bd", [128, NP, 32], BF16, isOutput=False)
    b1v = nc.declare_dram_parameter("b1v", [128, NP], F32, isOutput=False)
    w2v = nc.declare_dram_parameter("w2v", [128, NP], F32, isOutput=False)
    b2v = nc.declare_dram_parameter("b2v", [128, NP], F32, isOutput=False)
    b3b = nc.declare_dram_parameter("b3b", [128, STB], F32, isOutput=False)
    out = nc.declare_dram_parameter("out", [NST, BLK, STB], BF16, isOutput=True)

    with tile.TileContext(nc) as tc:
        with (
            tc.tile_pool(name="consts", bufs=1) as consts,
            tc.tile_pool(name="xs", bufs=5) as xs_pool,
            tc.tile_pool(name="h1", bufs=4) as h1_pool,
            tc.tile_pool(name="h2", bufs=8) as h2_pool,
            tc.tile_pool(name="osb", bufs=3) as osb_pool,
            tc.tile_pool(name="p1", bufs=6, space="PSUM") as p1_pool,
            tc.tile_pool(name="p2", bufs=2, space="PSUM") as p2_pool,
        ):
            w1bd_sb = consts.tile([KG, NG, 32], BF16)
            nc.scalar.dma_start(w1bd_sb[:], w1bd[:])
            w3bd_sb = consts.tile([128, NP, 32], BF16)
            nc.scalar.dma_start(w3bd_sb[:], w3bd[:])
            w2v_sb = consts.tile([128, NP], F32)
            nc.scalar.dma_start(w2v_sb[:], w2v[:])
            if not zero_b12:
                b1v_sb = consts.tile([128, NP], F32)
                nc.scalar.dma_start(b1v_sb[:], b1v[:])
                b2v_sb = consts.tile([128, NP], F32)
                nc.scalar.dma_start(b2v_sb[:], b2v[:])
            if not zero_b3:
                b3b_sb = consts.tile([128, STB], F32)
                nc.scalar.dma_start(b3b_sb[:], b3b[:])

            for st in range(NST):
                xsb = xs_pool.tile([KG, NG, STB], FP8E3, tag="xs")
                nc.sync.dma_start(xsb[:], xt[st])

                h2s = []
                for p in range(NP):
                    psum1 = p1_pool.tile([128, STB], F32, tag="p1")
                    for gl in range(NGP):
                        g = p * NGP + gl
                        nc.tensor.matmul(
                            psum1[gl * 32:(gl + 1) * 32, :],
                            w1bd_sb[:, g, :], xsb[:, g, :],
                            start=True, stop=True,
                            tile_position=(0, gl * 32),
                        )
                    h2 = h2_pool.tile([128, STB], BF16, tag="h2")
                    if zero_b12:
                        # h2 = relu(psum * max(W2,0)) -- exact when b1=b2=0
                        if p % 2 == 0:
                            nc.vector.tensor_scalar(
                                h2[:], psum1[:], w2v_sb[:, p:p + 1], 0.0,
                                op0=mybir.AluOpType.mult,
                                op1=mybir.AluOpType.max,
                            )
                        else:
                            nc.scalar.activation(
                                h2[:], psum1[:], RELU,
                                scale=w2v_sb[:, p:p + 1],
                            )
                    else:
                        h1 = h1_pool.tile([128, STB], BF16, tag="h1")
                        nc.vector.tensor_scalar(
                            h1[:], psum1[:], b1v_sb[:, p:p + 1], 0.0,
                            op0=mybir.AluOpType.add, op1=mybir.AluOpType.max,
                        )
                        nc.scalar.activation(
                            h2[:], h1[:], RELU,
                            bias=b2v_sb[:, p:p + 1], scale=w2v_sb[:, p:p + 1],
                        )
                    h2s.append(h2)

                psum2 = p2_pool.tile([128, STB], F32, tag="p2")
                for i in range(NBLK_ST):
                    for p in range(NP):
                        nc.tensor.matmul(
                            psum2[:, i * BLK + p * 32:i * BLK + (p + 1) * 32],
                            h2s[p][:, i * BLK:(i + 1) * BLK],
                            w3bd_sb[:, p, :],
                            start=True, stop=True,
                        )
                osb = osb_pool.tile([128, STB], BF16, tag="osb")
                if zero_b3:
                    if st % 2 == 0:
                        nc.vector.tensor_copy(osb[:], psum2[:])
                    else:
                        nc.scalar.activation(
                            osb[:], psum2[:],
                            mybir.ActivationFunctionType.Copy,
                        )
                else:
                    nc.vector.scalar_tensor_tensor(
                        osb[:], psum2[:], 1.0, b3b_sb[:],
                        op0=mybir.AluOpType.mult, op1=mybir.AluOpType.add,
                    )
                nc.scalar.dma_start(out[st], osb[:])

    nc.compile()
    return nc


def _softplus(v):
    return np.logaddexp(0.0, v)


def _spectral_op(W, sigma_scale, sigma_shift, alpha, residual_scale):
    U, S, Vh = np.linalg.svd(W, full_matrices=False)
    k = min(RANK, S.shape[-1])
    scale = _softplus(sigma_scale[:k])
    S_new = scale * S[:k] + sigma_shift[:k]
    if S.shape[-1] > k:
        S_new = np.concatenate([S_new, S[k:]], axis=-1)
    W_spec = (U * S_new[None, :]) @ Vh
    return alpha * W_spec + residual_scale * W


def _host_tables(weights_1, bias_1, bias_2, bias_3, contract_weights,
                 sigma_scale, sigma_shift, alpha, residual_scale):
    w1 = np.asarray(weights_1, np.float64)
    cw = np.asarray(contract_weights, np.float64)
    a = float(np.asarray(alpha).reshape(-1)[0])
    r = float(np.asarray(residual_scale).reshape(-1)[0])
    ss = np.asarray(sigma_scale, np.float64)
    sh = np.asarray(sigma_shift, np.float64)

    e = np.exp(cw - cw.max())
    w = e / e.sum()
    W1_c = np.einsum('m,mhd->hd', w, w1)
    W2 = _spectral_op(W1_c, ss, sh, a, r)
    W3 = _spectral_op(W2, ss, sh, a, r)

    b1 = np.asarray(bias_1, np.float64)[0]    # (H, D)
    b2 = np.asarray(bias_2, np.float64)[0]
    b3 = np.asarray(bias_3, np.float32).reshape(-1)
    zero_b12 = not (b1.any() or b2.any())
    zero_b3 = not b3.any()

    # mm1 stationary: rows (d8, m15), cols (d8, h), block-diagonal over d8
    w1bd = np.zeros((KG, NG, 32), np.float32)
    for g in range(NG):
        for d8 in range(GS):
            w1bd[d8 * M15:(d8 + 1) * M15, g, d8 * H:(d8 + 1) * H] = \
                w1[:, :, g * GS + d8]

    # mm2 moving operand: rows (gl, d8, h) [pack layout], cols (gl, d8)
    w3bd = np.zeros((128, NP, 32), np.float32)
    for p in range(NP):
        for gl in range(NGP):
            for d8 in range(GS):
                d = 32 * p + 8 * gl + d8
                w3bd[gl * 32 + d8 * H:gl * 32 + d8 * H + H, p, gl * 8 + d8] = W3[:, d]

    # b1/w2/b2: pack layout, partition = gl*32 + d8*4 + h
    pp = np.arange(128)
    gl_, d8_, h_ = pp // 32, (pp % 32) // 4, pp % 4
    b1v = np.zeros((128, NP), np.float32)
    w2v = np.zeros((128, NP), np.float32)
    b2v = np.zeros((128, NP), np.float32)
    for p in range(NP):
        d = 32 * p + 8 * gl_ + d8_
        b1v[:, p] = b1[h_, d]
        b2v[:, p] = b2[h_, d]
        w2v[:, p] = np.maximum(W2[h_, d], 0.0) if zero_b12 else W2[h_, d]

    b3b = np.broadcast_to(np.tile(b3, NBLK_ST), (128, STB)).copy()
    return dict(w1bd=w1bd.astype(BF), w3bd=w3bd.astype(BF),
                b1v=b1v, w2v=w2v, b2v=b2v, b3b=b3b), zero_b12, zero_b3


def _run(inputs, trace=False):
    x = np.asarray(inputs["pre_acts_history"], np.float32)
    tabs, zero_b12, zero_b3 = _host_tables(
        inputs["weights_1"], inputs["bias_1"], inputs["bias_2"],
        inputs["bias_3"], inputs["contract_weights"], inputs["sigma_scale"],
        inputs["sigma_shift"], inputs["alpha"], inputs["residual_scale"],
    )
    key = (zero_b12, zero_b3)
    if key not in _COMPILED:
        _COMPILED[key] = _build_nc(zero_b12, zero_b3)
    nc = _COMPILED[key]

    in_maps = []
    for c in range(N_CORES):
        # (BC, D, 15) -> (st, d8, m, g, b): row (d8*15+m) of group g = k-index
        # matching the w1bd stationary layout; each supertile contiguous.
        xc = x[c * BC:(c + 1) * BC, :, HIST - M15:].astype(F8)
        xc = xc.reshape(NST, STB, NG, GS, M15).transpose(0, 3, 4, 2, 1)
        m = {"xt": np.ascontiguousarray(xc).reshape(NST, KG, NG, STB)}
        m.update(tabs)
        in_maps.append(m)
    res = run_bass_kernel_spmd(nc, in_maps, core_ids=list(range(N_CORES)),
                               trace=trace)
    # out[st, p, (i, d)] -> row b = st*512 + i*128 + p
    outs = [res.results[c]["out"].astype(np.float32)
            .reshape(NST, BLK, NBLK_ST, BLK).transpose(0, 2, 1, 3)
            .reshape(BC, D) for c in range(N_CORES)]
    return np.concatenate(outs, axis=0), res


def kernel(**inputs) -> np.ndarray:
    out, _ = _run(inputs, trace=False)
    return out


def bench(inputs):
    """Run with NTFF tracing; returns (output, BassKernelResults)."""
    return _run(inputs, trace=True)
